# revision 2
# baseline (speedup 1.0000x reference)
"""Edge-parallel GNN kernel v3: sorted-src expansion + fp16 edge gather.

Reference computation (DTIConvGraph3):
    hs = atom_feats[src]; hd = atom_feats[dst]
    pre  = concat([hs, hd, bond]) @ W1.T + b1 + (hs+hd) @ W2.T + b2
    out  = leaky_relu(pre, 0.01)

Algebra: with W1 = [W1s | W1d | w1b],
    As = atom @ (W1s+W2).T + (b1+b2)     # per-node, bias folded in
    Ad = atom @ (W1d+W2).T               # per-node
    pre[e] = As[src[e]] + Ad[dst[e]] + bond[e]*w1b

Host (free): edges globally sorted by src, sharded 8 x 40000; each core's
edges tile into 2048-slot tiles whose src span <=126 consecutive nodes.
Output un-permuted + cast to f32 on host.

Device (fp16 everywhere, f32 PSUM):
  Phase 1: Ad table for all nodes -> HBM [NPAD, 128] f16 (gather source);
           As for each tile window -> SBUF window-major [128, nt*128],
           row 0 of each window block = w1b (bond weight row).
  Phase 2 per tile, edge-major (slot s = j*128+p):
    one-hot [w, e]: rows 1..127 staircase from run bounds, row 0 = bond
      values (DMA'd over). Generated via 2 ACT shifts (per-partition bias
      is ACT-native) + 2 const-scalar DVE compares (fast path; AP-scalar
      DVE ops measured 6x slower).
    16 matmuls/tile: psum[e,f] = oh_blk.T @ asw_blk -- includes As
      selection AND bond*w1b via row 0 in one pass.
    dst gather: per-edge 256B fp16 rows, non-transpose, 4 SWDGE queues,
      multi-packet (generation-bound floor ~150us/core).
    DVE adds psum+AdT, leaky-relu via const-scalar stt, fp16 DMA out.
"""

import sys

import numpy as np

if "/opt/trn_rl_repo" not in sys.path:
    sys.path.insert(0, "/opt/trn_rl_repo")

import concourse.bacc as bacc
import concourse.mybir as mybir
from concourse.bass_utils import run_bass_kernel_spmd
from concourse.tile import TileContext

N = 10000
D = 128
E = 320000
N_CORES = 8
EC = E // N_CORES          # 40000 edges per core
TILE_E = 2048
NBLKN = (N + 127) // 128   # 79 node blocks
NPAD = NBLKN * 128         # 10112
WMAX = 127                 # srcoff+1 in [1,127] -> span <= 126
NT_DEFAULT = (EC + TILE_E - 1) // TILE_E  # 20

NEG_SLOPE = 0.01

KERNEL_TRACE = False
LAST_EXEC_NS = None
LAST_RESULTS = None

_PROGRAM = {}


def _build_program(
    nt=NT_DEFAULT,
    repeat=1,
    gather_chunk=1024,
    gather_queues=4,
    single_packet=True,
    variant=0,  # 0=full, 1=no gather (memset), 6=gather-only
):
    f16 = mybir.dt.float16
    f32 = mybir.dt.float32
    i16 = mybir.dt.int16
    mult = mybir.AluOpType.mult
    is_ge = mybir.AluOpType.is_ge
    amax = mybir.AluOpType.max
    Ident = mybir.ActivationFunctionType.Identity

    nc = bacc.Bacc(
        "TRN2",
        target_bir_lowering=False,
        debug=False,
        num_devices=N_CORES,
        num_swdge_queues=gather_queues,
    )
    atomF = nc.declare_dram_parameter("atomF", [128, NPAD], f16, False)
    atomW = nc.declare_dram_parameter("atomW", [128, nt * 128], f16, False)
    wdT = nc.declare_dram_parameter("wdT", [128, 128], f16, False)
    wsT = nc.declare_dram_parameter("wsT", [128, 128], f16, False)
    bs = nc.declare_dram_parameter("bs", [1, 128], f16, False)
    w1b = nc.declare_dram_parameter("w1b", [1, 128], f16, False)
    iotaF = nc.declare_dram_parameter("iotaF", [128, TILE_E], f16, False)
    nstarts = nc.declare_dram_parameter("nstarts", [128, nt], f32, False)
    ends = nc.declare_dram_parameter("ends", [128, nt], f32, False)
    didx = nc.declare_dram_parameter(
        "didx", [128, nt * TILE_E // 16], i16, False
    )
    bondr = nc.declare_dram_parameter("bondr", [nt, TILE_E], f16, False)
    out = nc.declare_dram_parameter("out", [nt, 128, TILE_E], f16, True)
    adH = nc.dram_tensor("adH", [NPAD, 128], f16)

    with TileContext(nc) as tc:
        with (
            tc.tile_pool(name="const", bufs=1) as const,
            tc.tile_pool(name="ps", bufs=4, space="PSUM") as psum,
            tc.tile_pool(name="g", bufs=3) as g,
            tc.tile_pool(name="acc", bufs=3) as acc,
            tc.tile_pool(name="st", bufs=2) as st,
        ):
            atomF_sb = const.tile([128, NPAD], f16)
            nc.sync.dma_start(atomF_sb[:], atomF[:])
            atomW_sb = const.tile([128, nt * 128], f16)
            nc.sync.dma_start(atomW_sb[:], atomW[:])
            wdT_sb = const.tile([128, 128], f16)
            nc.sync.dma_start(wdT_sb[:], wdT[:])
            wsT_sb = const.tile([128, 128], f16)
            nc.sync.dma_start(wsT_sb[:], wsT[:])
            bs_sb = const.tile([1, 128], f16)
            nc.sync.dma_start(bs_sb[:], bs[:])
            w1b_sb = const.tile([1, 128], f16)
            nc.sync.dma_start(w1b_sb[:], w1b[:])
            iota_sb = const.tile([128, TILE_E], f16)
            nc.sync.dma_start(iota_sb[:], iotaF[:])
            nstarts_sb = const.tile([128, nt], f32)
            nc.sync.dma_start(nstarts_sb[:], nstarts[:])
            ends_sb = const.tile([128, nt], f32)
            nc.sync.dma_start(ends_sb[:], ends[:])
            didx_sb = const.tile([128, nt * TILE_E // 16], i16)
            nc.sync.dma_start(didx_sb[:], didx[:])
            ones_sb = const.tile([1, 128], f16)
            nc.vector.memset(ones_sb[:], 1.0)

            asw_sb = const.tile([128, nt * 128], f16)

            # ---- Phase 1a: Ad table -> HBM (row-major f16, gather source)
            for i in range(NBLKN):
                ps = psum.tile([128, 512], f32, tag="p2", name="ps")
                nc.tensor.matmul(
                    ps[:, 0:128],
                    atomF_sb[:, i * 128 : (i + 1) * 128],
                    wdT_sb[:],
                    start=True,
                    stop=True,
                )
                ab = st.tile([128, 128], f16, tag="ab")
                nc.scalar.copy(ab[:], ps[:, 0:128])
                nc.sync.dma_start(adH[i * 128 : (i + 1) * 128, :], ab[:])

            # ---- Phase 1b: As windows (+bias); row 0 of each block = w1b
            for t in range(nt):
                ps = psum.tile([128, 512], f32, tag="p2", name="ps")
                nc.tensor.matmul(
                    ps[:, 0:128],
                    atomW_sb[:, t * 128 : (t + 1) * 128],
                    wsT_sb[:],
                    start=True,
                    stop=False,
                )
                nc.tensor.matmul(
                    ps[:, 0:128], ones_sb[:], bs_sb[:], start=False, stop=True
                )
                nc.scalar.copy(asw_sb[:, t * 128 : (t + 1) * 128], ps[:, 0:128])
                nc.scalar.copy(
                    asw_sb[0:1, t * 128 : (t + 1) * 128], w1b_sb[:]
                )

            tc.strict_bb_all_engine_barrier()

            # ---- Phase 2
            import contextlib

            loop_cm = (
                tc.For_i(0, repeat, 1) if repeat > 1 else contextlib.nullcontext()
            )
            with loop_cm:
                ck = gather_chunk
                nck = TILE_E // ck
                gq = 0
                for t in range(nt):
                    # one-hot [w, e]: t1 = iota - start, t2 = end - iota (ACT,
                    # per-partition bias), oh = (t1>=0)*(t2>=0.5) (DVE, const
                    # scalars -> fast path). Row 0 (start=end=0) comes out all
                    # zero and is then overwritten with bond values by DMA.
                    t1 = g.tile([128, TILE_E], f16, tag="t1")
                    t2 = g.tile([128, TILE_E], f16, tag="t2")
                    oh = g.tile([128, TILE_E], f16, tag="oh")
                    if variant != 6:
                        nc.scalar.activation(
                            t1[:], iota_sb[:], Ident,
                            bias=nstarts_sb[:, t : t + 1], scale=1.0,
                        )
                        nc.scalar.activation(
                            t2[:], iota_sb[:], Ident,
                            bias=ends_sb[:, t : t + 1], scale=-1.0,
                        )
                        nc.vector.tensor_scalar(
                            oh[:], t1[:], 0.0, None, op0=is_ge
                        )
                        nc.vector.scalar_tensor_tensor(
                            oh[:], t2[:], 0.5, oh[:], op0=is_ge, op1=mult
                        )
                        nc.sync.dma_start(oh[0:1, :], bondr[t : t + 1, :])
                    # dst gather: per-edge 256B fp16 rows
                    adt = g.tile([128, TILE_E // 128, 128], f16, tag="adt")
                    if variant == 1:
                        nc.vector.memset(adt[:], 0.25)
                    else:
                        for c in range(nck):
                            idx0 = (t * TILE_E + c * ck) // 16
                            nc.gpsimd.dma_gather(
                                adt[
                                    :,
                                    c * (ck // 128) : (c + 1) * (ck // 128),
                                    :,
                                ],
                                adH[:],
                                didx_sb[:, idx0 : idx0 + ck // 16],
                                ck,
                                ck,
                                128,
                                elem_step=128,
                                single_packet=single_packet,
                                queue_num=gq % gather_queues,
                            )
                            gq += 1
                    if variant == 6:
                        ob = acc.tile(
                            [128, TILE_E // 128, 128], f16, tag="ob"
                        )
                        nc.vector.scalar_tensor_tensor(
                            ob[:], adt[:], NEG_SLOPE, adt[:],
                            op0=mult, op1=amax,
                        )
                        nc.sync.dma_start(
                            out[t, :, :],
                            ob[:].rearrange("p a b -> p (a b)"),
                        )
                        continue
                    pre = acc.tile([128, TILE_E // 128, 128], f16, tag="pre")
                    ob = acc.tile([128, TILE_E // 128, 128], f16, tag="ob")
                    for c in range(4):
                        pc = psum.tile([128, 512], f32, tag="p2", name="pc")
                        for b in range(4):
                            e0 = (c * 4 + b) * 128
                            nc.tensor.matmul(
                                pc[:, b * 128 : (b + 1) * 128],
                                oh[:, e0 : e0 + 128],
                                asw_sb[:, t * 128 : (t + 1) * 128],
                                start=True,
                                stop=True,
                            )
                        nc.vector.tensor_add(
                            pre[:, c * 4 : (c + 1) * 4, :].rearrange(
                                "p a b -> p (a b)"
                            ),
                            pc[:],
                            adt[:, c * 4 : (c + 1) * 4, :].rearrange(
                                "p a b -> p (a b)"
                            ),
                        )
                    nc.vector.scalar_tensor_tensor(
                        ob[:], pre[:], NEG_SLOPE, pre[:], op0=mult, op1=amax
                    )
                    nc.sync.dma_start(
                        out[t, :, :], ob[:].rearrange("p a b -> p (a b)")
                    )
    nc.compile()
    return nc


def _get_program(nt, **kw):
    key = (nt, tuple(sorted(kw.items())))
    if key not in _PROGRAM:
        _PROGRAM[key] = _build_program(nt=nt, **kw)
    return _PROGRAM[key]


def _shard_tiles(src_c, max_tile=TILE_E, wmax=WMAX):
    """Edge positions (sorted by src) -> list of (w0, n_edges) tiles with
    src span <= wmax-1 per tile."""
    n = len(src_c)
    ntiles_fast = (n + max_tile - 1) // max_tile
    ok = True
    for t in range(ntiles_fast):
        seg = src_c[t * max_tile : (t + 1) * max_tile]
        if len(seg) and seg[-1] - seg[0] > wmax - 1:
            ok = False
            break
    if ok:
        return [
            (int(src_c[t * max_tile]), min(max_tile, n - t * max_tile))
            for t in range(ntiles_fast)
        ]
    tiles = []
    i = 0
    while i < n:
        w0 = int(src_c[i])
        j_max = min(i + max_tile, n)
        j = int(np.searchsorted(src_c[i:j_max], w0 + wmax, side="left")) + i
        tiles.append((w0, j - i))
        i = j
    return tiles


def _host_prep(inputs):
    atom = np.asarray(inputs["atom_feats"], dtype=np.float32)
    bondf = np.asarray(inputs["bond_feats"], dtype=np.float32).reshape(-1)
    src = np.asarray(inputs["src"]).astype(np.int64)
    dst = np.asarray(inputs["dst"]).astype(np.int64)
    W1 = np.asarray(inputs["W1"], dtype=np.float32)
    b1 = np.asarray(inputs["b1"], dtype=np.float32)
    W2 = np.asarray(inputs["W2"], dtype=np.float32)
    b2 = np.asarray(inputs["b2"], dtype=np.float32)

    Ws = W1[:, :D] + W2
    Wd = W1[:, D : 2 * D] + W2
    w1b_v = W1[:, 2 * D]
    bias = b1 + b2

    order = np.argsort(src, kind="stable")
    atomT = np.zeros((128, NPAD), np.float16)
    atomT[:, :N] = atom.T.astype(np.float16)

    per_core = []
    nt_req = 0
    for c in range(N_CORES):
        eids = order[c * EC : (c + 1) * EC]
        tiles = _shard_tiles(src[eids])
        nt_req = max(nt_req, len(tiles))
        per_core.append((eids, tiles))
    nt = max(nt_req, NT_DEFAULT)

    iota = np.tile(np.arange(TILE_E, dtype=np.float16)[None, :], (128, 1))
    in_maps = []
    slot_maps = []
    for c in range(N_CORES):
        eids, tiles = per_core[c]
        src_c = src[eids]
        dst_c = dst[eids]
        bond_c = bondf[eids]

        atomW = np.zeros((128, nt * 128), np.float16)
        starts_a = np.zeros((128, nt), np.float32)
        ends_a = np.zeros((128, nt), np.float32)
        didx_a = np.zeros(nt * TILE_E, np.int64)
        bond_a = np.zeros(nt * TILE_E, np.float32)
        slot_map = np.full(nt * TILE_E, -1, np.int64)

        pos = 0
        for t, (w0, ne) in enumerate(tiles):
            sl = slice(pos, pos + ne)
            srcoff = src_c[sl] - w0 + 1  # rows 1..127 (row 0 = bond)
            assert srcoff.min() >= 1 and srcoff.max() <= 127
            starts_a[:, t] = np.searchsorted(srcoff, np.arange(128), "left")
            ends_a[:, t] = np.searchsorted(srcoff, np.arange(128), "right")
            didx_a[t * TILE_E : t * TILE_E + ne] = dst_c[sl]
            bond_a[t * TILE_E : t * TILE_E + ne] = bond_c[sl]
            slot_map[t * TILE_E : t * TILE_E + ne] = eids[sl]
            hi = min(w0 + 127, N)
            atomW[:, t * 128 + 1 : t * 128 + 1 + (hi - w0)] = atomT[:, w0:hi]
            pos += ne
        assert pos == len(eids)

        # wrap dst indices: position i -> partition i%16, col i//16
        # (per 1024-idx chunk)
        ch = didx_a.reshape(-1, 1024 // 16, 16).transpose(0, 2, 1)
        ch = ch.reshape(-1, 16, 64).transpose(1, 0, 2).reshape(16, -1)
        didx_w = np.tile(ch, (8, 1)).astype(np.int16)

        in_maps.append(
            {
                "atomF": atomT,
                "atomW": atomW,
                "wdT": np.ascontiguousarray(Wd.T).astype(np.float16),
                "wsT": np.ascontiguousarray(Ws.T).astype(np.float16),
                "bs": bias[None, :].astype(np.float16),
                "w1b": w1b_v[None, :].astype(np.float16),
                "iotaF": iota,
                "nstarts": -starts_a,
                "ends": ends_a,
                "didx": np.ascontiguousarray(didx_w),
                "bondr": bond_a.reshape(nt, TILE_E).astype(np.float16),
            }
        )
        slot_maps.append(slot_map)
    return nt, in_maps, slot_maps


def kernel(**inputs) -> np.ndarray:
    global LAST_EXEC_NS, LAST_RESULTS
    nt, in_maps, slot_maps = _host_prep(inputs)
    nc = _get_program(nt)
    res = run_bass_kernel_spmd(
        nc, in_maps, list(range(N_CORES)), trace=KERNEL_TRACE
    )
    LAST_EXEC_NS = res.exec_time_ns
    LAST_RESULTS = res
    result = np.zeros((E, D), np.float32)
    for c in range(N_CORES):
        o = np.asarray(res.results[c]["out"])  # [nt, 128, TILE_E] f16
        o = (
            o.reshape(-1, 128, TILE_E // 128, 128)
            .transpose(0, 2, 1, 3)
            .reshape(-1, 128)
        )
        sm = slot_maps[c]
        valid = sm >= 0
        result[sm[valid]] = o[valid].astype(np.float32)
    return result
